# revision 29
# baseline (speedup 1.0000x reference)
"""Trainium2 Bass kernel for nn_Attention2 (8-head encoder/decoder attention mix).

Reference computation (full batch B=4096):
    enc_h  = relu(encoder_input @ W_enc + b_enc)               [B, 1024]
    heads  = relu(einsum('bh,khd->kbd', enc_h, W_heads) + b_heads)  [8, B, 1024]
    dec_H  = relu(decoder_input @ W_dec + b_dec)               [B, 1024]
    scores = sum(heads * dec_H, axis=2)                        [8, B]
    attn   = softmax(scores.T, axis=1)                         [B, 8]
    out    = einsum('kbd,bk->bd', heads, attn)                 [B, 1024]

Sharding: pure data-parallel over batch across 8 NeuronCores (B_loc = 512
per core, params replicated, zero collectives).

Design (evolved from an all-bias-matmul baseline via perfetto/NTFF trace
analysis; all costs below measured on HW at the warm 2.4 GHz PE clock):

  - PE stream: 646 matmuls of [K<=128, M=128, N=512] at the 216 ns
    roofline spacing (vs 680 in the baseline). 608 do real work; 38
    inject biases via a K=128 matmul of ones/128 against a broadcast
    bias tile (the PSUM has_written mechanism means only a matmul can
    pre-load the accumulator).
  - Per [128 batch, 1024 hid] output tile (one 2-bank PSUM group, 16+1
    MMs = 3.67 us): the n=0 chunk's bias is added by a DVE
    tensor_tensor (PSUM + broadcast-bias SBUF tile -> SBUF, 686 ns), the
    n=1 chunk's by the bias matmul + relu straight from PSUM. The
    hybrid keeps DVE (tt-add 686 + score stt 1226 + out stt 1226 =
    3.1 us) and ScalarE (2 relus + exp = 1.7 us) under the PE budget --
    an all-DVE-bias variant saturated DVE and drained a ~20 us tail.
  - Stage D: score via one fused scalar_tensor_tensor (mult +
    free-dim accumulate); streaming normalizer-free softmax
    e = exp(score - 24) (scores measured in [14.2, 34.0]); out_acc
    (f32) accumulated in place by stt; h==0 initializes it via
    tensor_scalar (no memsets). Final divide via ScalarE Copy-with-
    scale (1/sum e), last tile on the by-then-idle DVE, chunked so the
    store DMA overlaps.
  - Head 7 uses bias matmuls for BOTH chunks so its four stage-D chains
    need no DVE tt-add: the tail after the last matmul is just the
    (7,3) chain (~6 us) instead of a ~15 us multi-chain drain.
  - DMA: host repacks every tensor so each load is one contiguous-row
    DMA ([p, k, c] layouts; W_heads as [128, 8192] per head, 16 KB
    rows). A queue sustains ~120 GB/s, so each head's 2 MB is split in
    1 MB halves on the sync and gpsimd queues (one head per 14.7 us
    period needs both); x_enc on the scalar queue. Issue order is
    need-time priority. Pool-gated W_heads DMAs (h>=4, which wait for
    an earlier head's buffer) live ONLY on non-compute queues --
    on the scalar queue they deadlock against the relus that retire
    the head they wait for. wh pool depth 4 heads.
  - 10 warmup matmuls on constants right after the preamble warm the
    HAM clock gate (a cold PE runs at 1.2 GHz for its first ~3.4 us)
    while the first input DMAs land.

Measured (core 0, NTFF profile, warm-clock runs): ~170 us HW exec,
rel err 4.0e-3 (bf16 matmuls, f32 stage-D). Baseline: 176-179 us.
Run-to-run variance: the PE clock occasionally sits at 2.0 GHz (P0
power state), which adds ~28 us to any version of this kernel; the
~7-11 us framework preamble (multi-core barrier) also varies.
"""

import os
import numpy as np
from contextlib import ExitStack

N_CORES = 8
ENC_DIM, DEC_DIM, HID, HEADS, BATCH = 1024, 512, 1024, 8, 4096
B_LOC = BATCH // N_CORES          # 512 batch rows per core
P = 128                           # SBUF partitions
NCHUNK = 512                      # matmul moving free-dim (one PSUM bank)
SCORE_SHIFT = 24.0                # scores measured in [14.2, 34.0]

_cache = {}


def _build():
    import concourse.tile as tile
    from concourse import bacc, mybir

    f32 = mybir.dt.float32
    bf16 = mybir.dt.bfloat16
    MM = bf16
    ST = f32                      # head storage dtype (score stt is f32-fast)
    Relu = mybir.ActivationFunctionType.Relu
    Exp = mybir.ActivationFunctionType.Exp
    Copy = mybir.ActivationFunctionType.Copy
    X = mybir.AxisListType.X
    mult = mybir.AluOpType.mult
    add = mybir.AluOpType.add

    KT_E = ENC_DIM // P           # 8 contraction tiles (enc dim)
    KT_H = HID // P               # 8 contraction tiles (hid dim)
    KT_D = DEC_DIM // P           # 4 contraction tiles (dec dim)
    MT = HID // P                 # 8 hid tiles (feature-major partitions)
    BT = B_LOC // P               # 4 batch tiles
    NC_H = HID // NCHUNK          # 2 moving chunks over hid
    HALF = HID // 2               # 512

    N_WARMUP = int(os.environ.get("BASS_WARMUP", "10"))

    nc = bacc.Bacc("TRN2", target_bir_lowering=False, debug=False,
                   num_devices=N_CORES)

    # host-repacked inputs (see build_in_maps)
    xe_r = nc.dram_tensor("x_enc_r", [2, P, (KT_E // 2) * B_LOC], MM,
                          kind="ExternalInput").ap()
    we_r = nc.dram_tensor("w_enc_r", [2, 2, P, (KT_E // 2) * HALF], MM,
                          kind="ExternalInput").ap()
    xd_r = nc.dram_tensor("x_dec_r", [P, KT_D * B_LOC], MM,
                          kind="ExternalInput").ap()
    wd_r = nc.dram_tensor("w_dec_r", [P, KT_D * HID], MM,
                          kind="ExternalInput").ap()
    wh_r = nc.dram_tensor("w_heads_r", [HEADS, P, KT_H * HID], MM,
                          kind="ExternalInput").ap()
    b_enc_pp = nc.dram_tensor("b_enc_pp", [P, MT], f32, kind="ExternalInput").ap()
    # broadcast bias tiles: bias replicated across the 128 partitions
    b_heads_bc = nc.dram_tensor("b_heads_bc", [HEADS, P, HID], MM,
                                kind="ExternalInput").ap()
    b_dec_bc = nc.dram_tensor("b_dec_bc", [P, HID], MM, kind="ExternalInput").ap()
    out_d = nc.dram_tensor("out", [B_LOC, HID], f32, kind="ExternalOutput").ap()

    with tile.TileContext(nc) as tc, ExitStack() as ctx:
        persist = ctx.enter_context(tc.tile_pool(name="persist", bufs=1))
        psums = ctx.enter_context(tc.tile_pool(name="psums", bufs=4, space="PSUM"))

        # --- constants / biases ---
        # ones/128 so a K=128 matmul against the full broadcast-bias tile
        # sums to exactly the bias
        ones_128 = persist.tile([P, P], MM, tag="ones128", name="ones128")
        nc.vector.memset(ones_128[:], 1.0 / P)
        warm_rhs = persist.tile([P, NCHUNK], MM, tag="wrhs", name="wrhs")
        nc.vector.memset(warm_rhs[:], 0.5)
        negC = persist.tile([P, 1], f32, tag="negC", name="negC")
        nc.vector.memset(negC[:], -SCORE_SHIFT)
        benc = persist.tile([P, MT], f32, tag="benc", name="benc")
        bh_bc = [persist.tile([P, HID], MM, tag=f"bhb{h}", name=f"bhb{h}")
                 for h in range(HEADS)]
        bd_bc = persist.tile([P, HID], MM, tag="bdb", name="bdb")

        # --- persistent activations ---
        ench = [persist.tile([P, B_LOC], MM, tag=f"ench{m}", name=f"ench{m}") for m in range(MT)]
        dec_bm = [persist.tile([P, HID], ST, tag=f"dec{b}", name=f"dec{b}") for b in range(BT)]
        e_all = [persist.tile([P, HEADS], f32, tag=f"eall{b}", name=f"eall{b}") for b in range(BT)]
        out_acc = [persist.tile([P, HID], ST, tag=f"oacc{b}", name=f"oacc{b}") for b in range(BT)]

        # ---- PE warmup: matmuls on constants so HAM un-throttles while the
        # first input DMAs are still in flight. Output never read.
        warm_ps = psums.tile([P, HID], f32, tag="mm", name="warm")
        for _ in range(N_WARMUP):
            nc.tensor.matmul(warm_ps[:, :NCHUNK], ones_128[:], warm_rhs[:],
                             start=True, stop=True)

        # head-weight tiles: one [128, 8*1024] tile per head, 3 in flight
        wh_pool = ctx.enter_context(tc.tile_pool(name="wh", bufs=4))

        with ExitStack() as actx:
            a_pool = actx.enter_context(tc.tile_pool(name="stageA", bufs=1))
            # xe[half][128, 4*512], we[wave][half][128, 4*512]
            xe = [a_pool.tile([P, (KT_E // 2) * B_LOC], MM, tag=f"xe{i}", name=f"xe{i}")
                  for i in range(2)]
            we = [[a_pool.tile([P, (KT_E // 2) * HALF], MM, tag=f"we{w}{i}", name=f"we{w}{i}")
                   for i in range(2)] for w in range(2)]
            xd = a_pool.tile([P, KT_D * B_LOC], MM, tag="xd", name="xd")
            wd = a_pool.tile([P, KT_D * HID], MM, tag="wd", name="wd")

            # ---- DMA issues, by queue and need-time priority. Each queue
            # sustains ~120 GB/s aggregate; early W_heads tiles are split in
            # two 1 MB halves on different queues so each lands in ~8 us.
            # Pool-blocked wh DMAs (h>=3, which wait for an earlier head's
            # buffer) go ONLY on non-compute queues (sync/gpsimd): on the
            # scalar queue they would deadlock against the relus that
            # retire the earlier head.
            WHH = (KT_H // 2) * HID   # half a head-weight tile (k=0..3)
            wh_tiles = [wh_pool.tile([P, KT_H * HID], MM, tag="whs", name=f"wh{h}")
                        for h in range(HEADS)]
            # sync queue (no compute): stage-A weights, a-halves of every
            # head, small biases. Each queue moves ~0.1 MB/us; one 2 MB head
            # per period (14.7 us) needs both queues carrying 1 MB each.
            nc.sync.dma_start(we[0][0][:], we_r[0, 0])
            nc.sync.dma_start(we[1][0][:], we_r[1, 0])
            nc.sync.dma_start(wh_tiles[0][:, :WHH], wh_r[0, :, :WHH])
            nc.sync.dma_start(bd_bc[:], b_dec_bc[:])
            nc.sync.dma_start(bh_bc[0][:], b_heads_bc[0])
            nc.sync.dma_start(wh_tiles[1][:, :WHH], wh_r[1, :, :WHH])
            nc.sync.dma_start(bh_bc[1][:], b_heads_bc[1])
            for h in range(2, HEADS):
                nc.sync.dma_start(wh_tiles[h][:, :WHH], wh_r[h, :, :WHH])
            # scalar queue: only never-blocking DMAs (pool-gated wh DMAs on a
            # compute queue would deadlock against the relus that retire the
            # earlier head)
            nc.scalar.dma_start(xe[0][:], xe_r[0])
            nc.scalar.dma_start(xe[1][:], xe_r[1])
            nc.scalar.dma_start(we[1][1][:], we_r[1, 1])
            nc.scalar.dma_start(benc[:], b_enc_pp[:])
            # gpsimd queue (no compute): dec inputs, b-halves, bias tiles
            nc.gpsimd.dma_start(we[0][1][:], we_r[0, 1])
            nc.gpsimd.dma_start(xd[:], xd_r[:])
            nc.gpsimd.dma_start(wd[:], wd_r[:])
            nc.gpsimd.dma_start(wh_tiles[0][:, WHH:], wh_r[0, :, WHH:])
            nc.gpsimd.dma_start(wh_tiles[1][:, WHH:], wh_r[1, :, WHH:])
            for h in range(2, 5):
                nc.gpsimd.dma_start(bh_bc[h][:], b_heads_bc[h])
            nc.gpsimd.dma_start(wh_tiles[2][:, WHH:], wh_r[2, :, WHH:])
            for h in range(5, HEADS):
                nc.gpsimd.dma_start(bh_bc[h][:], b_heads_bc[h])
            for h in range(3, HEADS):
                nc.gpsimd.dma_start(wh_tiles[h][:, WHH:], wh_r[h, :, WHH:])

            # ---- Stage C tile: hybrid bias (n=0 DVE tt-add, n=1 bias MM) --
            def stage_c_tile(b):
                ps = psums.tile([P, HID], f32, tag="mm", name="ps")
                for k in range(KT_D):
                    nc.tensor.matmul(
                        ps[:, :NCHUNK],
                        xd[:, k * B_LOC + b * P:k * B_LOC + (b + 1) * P],
                        wd[:, k * HID:k * HID + NCHUNK],
                        start=(k == 0), stop=(k == KT_D - 1))
                nc.tensor.matmul(ps[:, NCHUNK:], ones_128[:], bd_bc[:, NCHUNK:],
                                 start=True, stop=False)
                for k in range(KT_D):
                    nc.tensor.matmul(
                        ps[:, NCHUNK:],
                        xd[:, k * B_LOC + b * P:k * B_LOC + (b + 1) * P],
                        wd[:, k * HID + NCHUNK:(k + 1) * HID],
                        start=False, stop=(k == KT_D - 1))
                nc.vector.tensor_tensor(dec_bm[b][:, :NCHUNK], ps[:, :NCHUNK],
                                        bd_bc[:, :NCHUNK], op=add)
                nc.scalar.activation(dec_bm[b][:, :NCHUNK],
                                     dec_bm[b][:, :NCHUNK], Relu)
                nc.scalar.activation(dec_bm[b][:, NCHUNK:], ps[:, NCHUNK:], Relu)

            # ---- Stage A (enc trunk, feature-major), k-outer in 2 waves --
            for wave in range(2):
                pss = [psums.tile([P, HID], f32, tag="mm", name="ps")
                       for _ in range(MT // 2)]
                for k in range(KT_E):
                    xek = xe[k // 4][:, (k % 4) * B_LOC:(k % 4 + 1) * B_LOC]
                    wek = we[wave][k // 4][:, (k % 4) * HALF:(k % 4 + 1) * HALF]
                    for j in range(MT // 2):
                        nc.tensor.matmul(pss[j][:, :B_LOC],
                                         wek[:, j * P:(j + 1) * P], xek,
                                         start=(k == 0), stop=(k == KT_E - 1))
                for j in range(MT // 2):
                    m = wave * (MT // 2) + j
                    nc.scalar.activation(ench[m][:], pss[j][:, :B_LOC], Relu,
                                         bias=benc[:, m:m + 1], scale=1.0)
            for b in range(BT):
                stage_c_tile(b)

        # ---- Stage B + D: heads (batch-major), streaming softmax ----
        head_pool = ctx.enter_context(tc.tile_pool(name="head", bufs=3))
        scratch = ctx.enter_context(tc.tile_pool(name="scratch", bufs=4))

        schedule = [(h, b) for h in range(HEADS) for b in range(BT)]
        for h, b in schedule:
                wh = wh_tiles[h]
                last = h == HEADS - 1 and b == BT - 1
                tail2 = h == HEADS - 1
                head_t = head_pool.tile([P, HID], ST, tag=f"head{b}", name=f"head{b}")
                ps = psums.tile([P, HID], f32, tag="mm", name="ps")
                # n=0 chunk: bias via DVE tt-add afterwards -- except for the
                # final two groups, where a bias matmul keeps the tail chain
                # off the (still-draining) DVE queue
                if tail2:
                    nc.tensor.matmul(ps[:, :NCHUNK], ones_128[:],
                                     bh_bc[h][:, :NCHUNK], start=True, stop=False)
                for k in range(KT_H):
                    nc.tensor.matmul(
                        ps[:, :NCHUNK], ench[k][:, b * P:(b + 1) * P],
                        wh[:, k * HID:k * HID + NCHUNK],
                        start=(not tail2 and k == 0), stop=(k == KT_H - 1))
                # n=1 chunk: bias injected by a K=128 matmul (ones/128 against
                # the broadcast-bias tile), relu straight from PSUM
                nc.tensor.matmul(ps[:, NCHUNK:], ones_128[:],
                                 bh_bc[h][:, NCHUNK:], start=True, stop=False)
                for k in range(KT_H):
                    nc.tensor.matmul(
                        ps[:, NCHUNK:], ench[k][:, b * P:(b + 1) * P],
                        wh[:, k * HID + NCHUNK:(k + 1) * HID],
                        start=False, stop=(k == KT_H - 1))
                prod = scratch.tile([P, HID], ST, tag="prod", name="prod")
                s_col = scratch.tile([P, 1], f32, tag="scol", name="scol")
                # n=0: tt-add + in-place relu (or relu from psum for the
                # bias-matmul tail groups); n=1: relu from psum
                if tail2:
                    nc.scalar.activation(head_t[:, :NCHUNK], ps[:, :NCHUNK],
                                         Relu)
                else:
                    nc.vector.tensor_tensor(head_t[:, :NCHUNK], ps[:, :NCHUNK],
                                            bh_bc[h][:, :NCHUNK], op=add)
                    nc.scalar.activation(head_t[:, :NCHUNK], head_t[:, :NCHUNK],
                                         Relu)
                nc.scalar.activation(head_t[:, NCHUNK:], ps[:, NCHUNK:], Relu)
                # score: s_col = sum_hid(head * dec)  (fused mult+accum)
                if not last:
                    nc.vector.scalar_tensor_tensor(
                        prod[:], head_t[:], 1.0, dec_bm[b][:],
                        op0=mult, op1=mult, accum_out=s_col[:])
                else:
                    # last head: half-tile ops so the kernel tail pipelines
                    s_half = scratch.tile([P, 1], f32, tag="shalf", name="shalf")
                    nc.vector.scalar_tensor_tensor(
                        prod[:, :NCHUNK], head_t[:, :NCHUNK], 1.0,
                        dec_bm[b][:, :NCHUNK], op0=mult, op1=mult,
                        accum_out=s_half[:])
                    nc.vector.scalar_tensor_tensor(
                        prod[:, NCHUNK:], head_t[:, NCHUNK:], 1.0,
                        dec_bm[b][:, NCHUNK:], op0=mult, op1=mult,
                        accum_out=s_col[:])
                    nc.vector.tensor_add(s_col[:], s_col[:], s_half[:])
                # e = exp(score - C)
                nc.scalar.activation(e_all[b][:, h:h + 1], s_col[:], Exp,
                                     bias=negC[:], scale=1.0)
                # out_acc += e_h * head (DVE fused stt, in-place accumulate);
                # h==0 initializes via tensor_scalar (no memset, no add)
                if h == 0:
                    nc.vector.tensor_scalar(
                        out_acc[b][:], head_t[:], e_all[b][:, h:h + 1], None,
                        op0=mult)
                elif not last:
                    nc.vector.scalar_tensor_tensor(
                        out_acc[b][:], head_t[:], e_all[b][:, h:h + 1],
                        out_acc[b][:], op0=mult, op1=add)
                else:
                    nc.vector.scalar_tensor_tensor(
                        out_acc[b][:], head_t[:], e_all[b][:, h:h + 1],
                        out_acc[b][:], op0=mult, op1=add)

        # ---- Final: divide by sum of exps, write out ----
        fin = ctx.enter_context(tc.tile_pool(name="fin", bufs=2))
        for b in range(BT):
            s_sum = fin.tile([P, 1], f32, tag="ssum", name="ssum")
            rinv = fin.tile([P, 1], f32, tag="rinv", name="rinv")
            nc.vector.reduce_sum(s_sum[:], e_all[b][:], axis=X)
            nc.vector.reciprocal(rinv[:], s_sum[:])
            out_f = fin.tile([P, HID], f32, tag="outf", name="outf")
            if b < BT - 1:
                nc.scalar.activation(out_f[:], out_acc[b][:], Copy, scale=rinv[:])
                nc.sync.dma_start(out_d[b * P:(b + 1) * P, :], out_f[:])
            else:
                # last tile: DVE is idle by now and faster than the ScalarE
                # copy; chunked so the first DMA overlaps the second multiply
                for n in range(NC_H):
                    ncol = slice(n * NCHUNK, (n + 1) * NCHUNK)
                    nc.vector.tensor_scalar_mul(out_f[:, ncol],
                                                out_acc[b][:, ncol], rinv[:])
                    nc.sync.dma_start(out_d[b * P:(b + 1) * P, ncol],
                                      out_f[:, ncol])

    nc.compile()
    return nc


def _get_nc():
    if "nc" not in _cache:
        _cache["nc"] = _build()
    return _cache["nc"]


def build_in_maps(encoder_input, decoder_input, W_enc, b_enc, W_heads,
                  b_heads, W_dec, b_dec):
    import ml_dtypes
    bf = ml_dtypes.bfloat16
    f32c = lambda a: np.asarray(a, dtype=np.float32)
    cast = lambda a: np.ascontiguousarray(a, dtype=np.float32).astype(bf)

    KT_E, KT_D, KT_H = ENC_DIM // P, DEC_DIM // P, HID // P

    xeT = f32c(encoder_input).T                     # [1024, 4096]
    xdT = f32c(decoder_input).T                     # [512, 4096]
    W_enc = f32c(W_enc)                             # [1024, 1024]
    W_dec = f32c(W_dec)                             # [512, 1024]
    W_heads = f32c(W_heads)                         # [8, 1024, 1024]

    # w_enc_r[w, i][p, k4*512 + c] = W_enc[(i*4+k4)*128+p, w*512 + c]
    we4 = W_enc.reshape(KT_E, P, 2, HID // 2)       # [k, p, wave, c]
    we_r = np.zeros((2, 2, P, (KT_E // 2) * (HID // 2)), np.float32)
    for w in range(2):
        for i in range(2):
            blk = we4[i * 4:(i + 1) * 4, :, w, :]   # [4k, 128, 512]
            we_r[w, i] = blk.transpose(1, 0, 2).reshape(P, -1)
    xe4 = xeT.reshape(KT_E, P, BATCH)
    wd4 = W_dec.reshape(KT_D, P, HID)
    wd_r = np.ascontiguousarray(wd4.transpose(1, 0, 2).reshape(P, -1))
    wh4 = W_heads.reshape(HEADS, KT_H, P, HID)
    wh_r = np.ascontiguousarray(wh4.transpose(0, 2, 1, 3).reshape(HEADS, P, -1))

    bh_bc = np.broadcast_to(f32c(b_heads)[:, None, :], (HEADS, P, HID))
    bd_bc = np.broadcast_to(f32c(b_dec)[None, :], (P, HID))
    shared = {
        "w_enc_r": cast(we_r),
        "b_enc_pp": np.ascontiguousarray(f32c(b_enc).reshape(HID // P, P).T),
        "w_heads_r": cast(wh_r),
        "b_heads_bc": cast(bh_bc),
        "w_dec_r": cast(wd_r),
        "b_dec_bc": cast(bd_bc),
    }
    xd4 = xdT.reshape(KT_D, P, BATCH)
    in_maps = []
    for c in range(N_CORES):
        sl = slice(c * B_LOC, (c + 1) * B_LOC)
        m = dict(shared)
        xe_c = xe4[:, :, sl]                        # [8k, 128, 512]
        m["x_enc_r"] = cast(np.stack(
            [xe_c[i * 4:(i + 1) * 4].transpose(1, 0, 2).reshape(P, -1)
             for i in range(2)]))
        m["x_dec_r"] = cast(xd4[:, :, sl].transpose(1, 0, 2).reshape(P, -1))
        in_maps.append(m)
    return in_maps


def kernel(encoder_input, decoder_input, W_enc, b_enc, W_heads, b_heads,
           W_dec, b_dec):
    from concourse.bass_utils import run_bass_kernel_spmd

    nc = _get_nc()
    in_maps = build_in_maps(encoder_input, decoder_input, W_enc, b_enc,
                            W_heads, b_heads, W_dec, b_dec)
    res = run_bass_kernel_spmd(nc, in_maps, list(range(N_CORES)))
    out = np.concatenate([res.results[c]["out"] for c in range(N_CORES)], axis=0)
    return out.astype(np.float32)


# revision 30
# speedup vs baseline: 1.0543x; 1.0543x over previous
"""Trainium2 Bass kernel for nn_Attention2 (8-head encoder/decoder attention mix).

Reference computation (full batch B=4096):
    enc_h  = relu(encoder_input @ W_enc + b_enc)               [B, 1024]
    heads  = relu(einsum('bh,khd->kbd', enc_h, W_heads) + b_heads)  [8, B, 1024]
    dec_H  = relu(decoder_input @ W_dec + b_dec)               [B, 1024]
    scores = sum(heads * dec_H, axis=2)                        [8, B]
    attn   = softmax(scores.T, axis=1)                         [B, 8]
    out    = einsum('kbd,bk->bd', heads, attn)                 [B, 1024]

Sharding: pure data-parallel over batch across 8 NeuronCores (B_loc = 512
per core, params replicated, zero collectives).

Design (evolved from an all-bias-matmul baseline via perfetto/NTFF trace
analysis; all costs below measured on HW at the warm 2.4 GHz PE clock):

  - PE stream: 646 matmuls of [K<=128, M=128, N=512] at the 216 ns
    roofline spacing (vs 680 in the baseline). 608 do real work; 38
    inject biases via a K=128 matmul of ones/128 against a broadcast
    bias tile (the PSUM has_written mechanism means only a matmul can
    pre-load the accumulator).
  - Per [128 batch, 1024 hid] output tile (one 2-bank PSUM group, 16+1
    MMs = 3.67 us): the n=0 chunk's bias is added by a DVE
    tensor_tensor (PSUM + broadcast-bias SBUF tile -> SBUF, 686 ns), the
    n=1 chunk's by the bias matmul + relu straight from PSUM. The
    hybrid keeps DVE (tt-add 686 + score stt 1226 + out stt 1226 =
    3.1 us) and ScalarE (2 relus + exp = 1.7 us) under the PE budget --
    an all-DVE-bias variant saturated DVE and drained a ~20 us tail.
  - Stage D: score via one fused scalar_tensor_tensor (mult +
    free-dim accumulate); streaming normalizer-free softmax
    e = exp(score - 24) (scores measured in [14.2, 34.0]); out_acc
    (f32) accumulated in place by stt; h==0 initializes it via
    tensor_scalar (no memsets). Final divide via ScalarE Copy-with-
    scale (1/sum e), last tile on the by-then-idle DVE, chunked so the
    store DMA overlaps.
  - Head 7 uses bias matmuls for BOTH chunks so its four stage-D chains
    need no DVE tt-add: the tail after the last matmul is just the
    (7,3) chain (~6 us) instead of a ~15 us multi-chain drain.
  - DMA: host repacks every tensor so each load is one contiguous-row
    DMA ([p, k, c] layouts; W_heads as [128, 8192] per head, 16 KB
    rows). A queue sustains ~120 GB/s, so each head's 2 MB is split in
    1 MB halves on the sync and gpsimd queues (one head per 14.7 us
    period needs both); x_enc on the scalar queue. Issue order is
    need-time priority. Pool-gated W_heads DMAs (h>=4, which wait for
    an earlier head's buffer) live ONLY on non-compute queues --
    on the scalar queue they deadlock against the relus that retire
    the head they wait for. wh pool depth 4 heads.
  - 10 warmup matmuls on constants right after the preamble warm the
    HAM clock gate (a cold PE runs at 1.2 GHz for its first ~3.4 us)
    while the first input DMAs land.

Measured (core 0, NTFF profile, warm-clock runs): ~170 us HW exec,
rel err 4.0e-3 (bf16 matmuls, f32 stage-D). Baseline: 176-179 us.
Run-to-run variance: the PE clock occasionally sits at 2.0 GHz (P0
power state), which adds ~28 us to any version of this kernel; the
~7-11 us framework preamble (multi-core barrier) also varies.
"""

import os
import numpy as np
from contextlib import ExitStack

N_CORES = 8
ENC_DIM, DEC_DIM, HID, HEADS, BATCH = 1024, 512, 1024, 8, 4096
B_LOC = BATCH // N_CORES          # 512 batch rows per core
P = 128                           # SBUF partitions
NCHUNK = 512                      # matmul moving free-dim (one PSUM bank)
SCORE_SHIFT = 24.0                # scores measured in [14.2, 34.0]

_cache = {}


def _build():
    import concourse.tile as tile
    from concourse import bacc, mybir

    f32 = mybir.dt.float32
    bf16 = mybir.dt.bfloat16
    MM = bf16
    ST = f32                      # head storage dtype (score stt is f32-fast)
    Relu = mybir.ActivationFunctionType.Relu
    Exp = mybir.ActivationFunctionType.Exp
    Copy = mybir.ActivationFunctionType.Copy
    X = mybir.AxisListType.X
    mult = mybir.AluOpType.mult
    add = mybir.AluOpType.add

    KT_E = ENC_DIM // P           # 8 contraction tiles (enc dim)
    KT_H = HID // P               # 8 contraction tiles (hid dim)
    KT_D = DEC_DIM // P           # 4 contraction tiles (dec dim)
    MT = HID // P                 # 8 hid tiles (feature-major partitions)
    BT = B_LOC // P               # 4 batch tiles
    NC_H = HID // NCHUNK          # 2 moving chunks over hid
    HALF = HID // 2               # 512

    N_WARMUP = int(os.environ.get("BASS_WARMUP", "10"))

    nc = bacc.Bacc("TRN2", target_bir_lowering=False, debug=False,
                   num_devices=N_CORES)

    # host-repacked inputs (see build_in_maps)
    xe_r = nc.dram_tensor("x_enc_r", [2, P, (KT_E // 2) * B_LOC], MM,
                          kind="ExternalInput").ap()
    we_r = nc.dram_tensor("w_enc_r", [2, 2, P, (KT_E // 2) * HALF], MM,
                          kind="ExternalInput").ap()
    xd_r = nc.dram_tensor("x_dec_r", [P, KT_D * B_LOC], MM,
                          kind="ExternalInput").ap()
    wd_r = nc.dram_tensor("w_dec_r", [P, KT_D * HID], MM,
                          kind="ExternalInput").ap()
    wh_r = nc.dram_tensor("w_heads_r", [HEADS, P, KT_H * HID], MM,
                          kind="ExternalInput").ap()
    b_enc_pp = nc.dram_tensor("b_enc_pp", [P, MT], f32, kind="ExternalInput").ap()
    # broadcast bias tiles: bias replicated across the 128 partitions
    b_heads_bc = nc.dram_tensor("b_heads_bc", [HEADS, P, HID], MM,
                                kind="ExternalInput").ap()
    b_dec_bc = nc.dram_tensor("b_dec_bc", [P, HID], MM, kind="ExternalInput").ap()
    out_d = nc.dram_tensor("out", [B_LOC, HID], f32, kind="ExternalOutput").ap()

    with tile.TileContext(nc) as tc, ExitStack() as ctx:
        persist = ctx.enter_context(tc.tile_pool(name="persist", bufs=1))
        psums = ctx.enter_context(tc.tile_pool(name="psums", bufs=4, space="PSUM"))

        # --- constants / biases ---
        # ones/128 so a K=128 matmul against the full broadcast-bias tile
        # sums to exactly the bias
        ones_128 = persist.tile([P, P], MM, tag="ones128", name="ones128")
        nc.vector.memset(ones_128[:], 1.0 / P)
        warm_rhs = persist.tile([P, NCHUNK], MM, tag="wrhs", name="wrhs")
        nc.vector.memset(warm_rhs[:], 0.5)
        negC = persist.tile([P, 1], f32, tag="negC", name="negC")
        nc.vector.memset(negC[:], -SCORE_SHIFT)
        benc = persist.tile([P, MT], f32, tag="benc", name="benc")
        bh_bc = [persist.tile([P, HID], MM, tag=f"bhb{h}", name=f"bhb{h}")
                 for h in range(HEADS)]
        bd_bc = persist.tile([P, HID], MM, tag="bdb", name="bdb")

        # --- persistent activations ---
        ench = [persist.tile([P, B_LOC], MM, tag=f"ench{m}", name=f"ench{m}") for m in range(MT)]
        dec_bm = [persist.tile([P, HID], ST, tag=f"dec{b}", name=f"dec{b}") for b in range(BT)]
        e_all = [persist.tile([P, HEADS], f32, tag=f"eall{b}", name=f"eall{b}") for b in range(BT)]
        out_acc = [persist.tile([P, HID], ST, tag=f"oacc{b}", name=f"oacc{b}") for b in range(BT)]

        # ---- PE warmup: matmuls on constants so HAM un-throttles while the
        # first input DMAs are still in flight. Output never read.
        warm_ps = psums.tile([P, HID], f32, tag="mm", name="warm")
        for _ in range(N_WARMUP):
            nc.tensor.matmul(warm_ps[:, :NCHUNK], ones_128[:], warm_rhs[:],
                             start=True, stop=True)

        # head-weight tiles: one [128, 8*1024] tile per head, 3 in flight
        wh_pool = ctx.enter_context(tc.tile_pool(name="wh", bufs=4))

        with ExitStack() as actx:
            a_pool = actx.enter_context(tc.tile_pool(name="stageA", bufs=1))
            # xe[half][128, 4*512], we[wave][half][128, 4*512]
            xe = [a_pool.tile([P, (KT_E // 2) * B_LOC], MM, tag=f"xe{i}", name=f"xe{i}")
                  for i in range(2)]
            we = [[a_pool.tile([P, (KT_E // 2) * HALF], MM, tag=f"we{w}{i}", name=f"we{w}{i}")
                   for i in range(2)] for w in range(2)]
            xd = a_pool.tile([P, KT_D * B_LOC], MM, tag="xd", name="xd")
            wd = a_pool.tile([P, KT_D * HID], MM, tag="wd", name="wd")

            # ---- DMA issues, by queue and need-time priority. Each queue
            # sustains ~120 GB/s aggregate; early W_heads tiles are split in
            # two 1 MB halves on different queues so each lands in ~8 us.
            # Pool-blocked wh DMAs (h>=3, which wait for an earlier head's
            # buffer) go ONLY on non-compute queues (sync/gpsimd): on the
            # scalar queue they would deadlock against the relus that
            # retire the earlier head.
            WHH = (KT_H // 2) * HID   # half a head-weight tile (k=0..3)
            wh_tiles = [wh_pool.tile([P, KT_H * HID], MM, tag="whs", name=f"wh{h}")
                        for h in range(HEADS)]
            # sync queue (no compute): stage-A weights, a-halves of every
            # head, small biases. Each queue moves ~0.1 MB/us; one 2 MB head
            # per period (14.7 us) needs both queues carrying 1 MB each.
            nc.sync.dma_start(we[0][0][:], we_r[0, 0])
            nc.sync.dma_start(we[1][0][:], we_r[1, 0])
            nc.sync.dma_start(we[1][1][:], we_r[1, 1])
            nc.sync.dma_start(wh_tiles[0][:, :WHH], wh_r[0, :, :WHH])
            nc.sync.dma_start(bd_bc[:], b_dec_bc[:])
            nc.sync.dma_start(bh_bc[0][:], b_heads_bc[0])
            nc.sync.dma_start(wh_tiles[1][:, :WHH], wh_r[1, :, :WHH])
            nc.sync.dma_start(bh_bc[1][:], b_heads_bc[1])
            for h in range(2, HEADS):
                nc.sync.dma_start(wh_tiles[h][:, :WHH], wh_r[h, :, :WHH])
            # scalar queue: only never-blocking DMAs (pool-gated wh DMAs on a
            # compute queue would deadlock against the relus that retire the
            # earlier head)
            nc.scalar.dma_start(xe[0][:], xe_r[0])
            nc.scalar.dma_start(xe[1][:], xe_r[1])
            nc.scalar.dma_start(benc[:], b_enc_pp[:])
            # gpsimd queue (no compute): dec inputs, b-halves, bias tiles
            nc.gpsimd.dma_start(we[0][1][:], we_r[0, 1])
            nc.gpsimd.dma_start(xd[:], xd_r[:])
            nc.gpsimd.dma_start(wd[:], wd_r[:])
            nc.gpsimd.dma_start(wh_tiles[0][:, WHH:], wh_r[0, :, WHH:])
            nc.gpsimd.dma_start(wh_tiles[1][:, WHH:], wh_r[1, :, WHH:])
            for h in range(2, 5):
                nc.gpsimd.dma_start(bh_bc[h][:], b_heads_bc[h])
            nc.gpsimd.dma_start(wh_tiles[2][:, WHH:], wh_r[2, :, WHH:])
            for h in range(5, HEADS):
                nc.gpsimd.dma_start(bh_bc[h][:], b_heads_bc[h])
            for h in range(3, HEADS):
                nc.gpsimd.dma_start(wh_tiles[h][:, WHH:], wh_r[h, :, WHH:])

            # ---- Stage C tile: hybrid bias (n=0 DVE tt-add, n=1 bias MM) --
            def stage_c_tile(b):
                ps = psums.tile([P, HID], f32, tag="mm", name="ps")
                for k in range(KT_D):
                    nc.tensor.matmul(
                        ps[:, :NCHUNK],
                        xd[:, k * B_LOC + b * P:k * B_LOC + (b + 1) * P],
                        wd[:, k * HID:k * HID + NCHUNK],
                        start=(k == 0), stop=(k == KT_D - 1))
                nc.tensor.matmul(ps[:, NCHUNK:], ones_128[:], bd_bc[:, NCHUNK:],
                                 start=True, stop=False)
                for k in range(KT_D):
                    nc.tensor.matmul(
                        ps[:, NCHUNK:],
                        xd[:, k * B_LOC + b * P:k * B_LOC + (b + 1) * P],
                        wd[:, k * HID + NCHUNK:(k + 1) * HID],
                        start=False, stop=(k == KT_D - 1))
                nc.vector.tensor_tensor(dec_bm[b][:, :NCHUNK], ps[:, :NCHUNK],
                                        bd_bc[:, :NCHUNK], op=add)
                nc.scalar.activation(dec_bm[b][:, :NCHUNK],
                                     dec_bm[b][:, :NCHUNK], Relu)
                nc.scalar.activation(dec_bm[b][:, NCHUNK:], ps[:, NCHUNK:], Relu)

            # ---- Stage A (enc trunk, feature-major), k-outer in 2 waves --
            for wave in range(2):
                pss = [psums.tile([P, HID], f32, tag="mm", name="ps")
                       for _ in range(MT // 2)]
                for k in range(KT_E):
                    xek = xe[k // 4][:, (k % 4) * B_LOC:(k % 4 + 1) * B_LOC]
                    wek = we[wave][k // 4][:, (k % 4) * HALF:(k % 4 + 1) * HALF]
                    for j in range(MT // 2):
                        nc.tensor.matmul(pss[j][:, :B_LOC],
                                         wek[:, j * P:(j + 1) * P], xek,
                                         start=(k == 0), stop=(k == KT_E - 1))
                for j in range(MT // 2):
                    m = wave * (MT // 2) + j
                    nc.scalar.activation(ench[m][:], pss[j][:, :B_LOC], Relu,
                                         bias=benc[:, m:m + 1], scale=1.0)
            for b in range(BT):
                stage_c_tile(b)

        # ---- Stage B + D: heads (batch-major), streaming softmax ----
        head_pool = ctx.enter_context(tc.tile_pool(name="head", bufs=3))
        scratch = ctx.enter_context(tc.tile_pool(name="scratch", bufs=4))

        schedule = [(h, b) for h in range(HEADS) for b in range(BT)]
        for h, b in schedule:
                wh = wh_tiles[h]
                last = h == HEADS - 1 and b == BT - 1
                tail2 = h == HEADS - 1
                head_t = head_pool.tile([P, HID], ST, tag=f"head{b}", name=f"head{b}")
                ps = psums.tile([P, HID], f32, tag="mm", name="ps")
                # n=0 chunk: bias via DVE tt-add afterwards -- except for the
                # final two groups, where a bias matmul keeps the tail chain
                # off the (still-draining) DVE queue
                if tail2:
                    nc.tensor.matmul(ps[:, :NCHUNK], ones_128[:],
                                     bh_bc[h][:, :NCHUNK], start=True, stop=False)
                for k in range(KT_H):
                    nc.tensor.matmul(
                        ps[:, :NCHUNK], ench[k][:, b * P:(b + 1) * P],
                        wh[:, k * HID:k * HID + NCHUNK],
                        start=(not tail2 and k == 0), stop=(k == KT_H - 1))
                # n=1 chunk: bias injected by a K=128 matmul (ones/128 against
                # the broadcast-bias tile), relu straight from PSUM
                nc.tensor.matmul(ps[:, NCHUNK:], ones_128[:],
                                 bh_bc[h][:, NCHUNK:], start=True, stop=False)
                for k in range(KT_H):
                    nc.tensor.matmul(
                        ps[:, NCHUNK:], ench[k][:, b * P:(b + 1) * P],
                        wh[:, k * HID + NCHUNK:(k + 1) * HID],
                        start=False, stop=(k == KT_H - 1))
                prod = scratch.tile([P, HID], ST, tag="prod", name="prod")
                s_col = scratch.tile([P, 1], f32, tag="scol", name="scol")
                # n=0: tt-add + in-place relu (or relu from psum for the
                # bias-matmul tail groups); n=1: relu from psum
                if tail2:
                    nc.scalar.activation(head_t[:, :NCHUNK], ps[:, :NCHUNK],
                                         Relu)
                else:
                    nc.vector.tensor_tensor(head_t[:, :NCHUNK], ps[:, :NCHUNK],
                                            bh_bc[h][:, :NCHUNK], op=add)
                    nc.scalar.activation(head_t[:, :NCHUNK], head_t[:, :NCHUNK],
                                         Relu)
                nc.scalar.activation(head_t[:, NCHUNK:], ps[:, NCHUNK:], Relu)
                # score: s_col = sum_hid(head * dec)  (fused mult+accum)
                if not last:
                    nc.vector.scalar_tensor_tensor(
                        prod[:], head_t[:], 1.0, dec_bm[b][:],
                        op0=mult, op1=mult, accum_out=s_col[:])
                else:
                    # last head: half-tile ops so the kernel tail pipelines
                    s_half = scratch.tile([P, 1], f32, tag="shalf", name="shalf")
                    nc.vector.scalar_tensor_tensor(
                        prod[:, :NCHUNK], head_t[:, :NCHUNK], 1.0,
                        dec_bm[b][:, :NCHUNK], op0=mult, op1=mult,
                        accum_out=s_half[:])
                    nc.vector.scalar_tensor_tensor(
                        prod[:, NCHUNK:], head_t[:, NCHUNK:], 1.0,
                        dec_bm[b][:, NCHUNK:], op0=mult, op1=mult,
                        accum_out=s_col[:])
                    nc.vector.tensor_add(s_col[:], s_col[:], s_half[:])
                # e = exp(score - C)
                nc.scalar.activation(e_all[b][:, h:h + 1], s_col[:], Exp,
                                     bias=negC[:], scale=1.0)
                # out_acc += e_h * head (DVE fused stt, in-place accumulate);
                # h==0 initializes via tensor_scalar (no memset, no add)
                if h == 0:
                    nc.vector.tensor_scalar(
                        out_acc[b][:], head_t[:], e_all[b][:, h:h + 1], None,
                        op0=mult)
                elif not last:
                    nc.vector.scalar_tensor_tensor(
                        out_acc[b][:], head_t[:], e_all[b][:, h:h + 1],
                        out_acc[b][:], op0=mult, op1=add)
                else:
                    nc.vector.scalar_tensor_tensor(
                        out_acc[b][:], head_t[:], e_all[b][:, h:h + 1],
                        out_acc[b][:], op0=mult, op1=add)

        # ---- Final: divide by sum of exps, write out ----
        fin = ctx.enter_context(tc.tile_pool(name="fin", bufs=2))
        for b in range(BT):
            s_sum = fin.tile([P, 1], f32, tag="ssum", name="ssum")
            rinv = fin.tile([P, 1], f32, tag="rinv", name="rinv")
            nc.vector.reduce_sum(s_sum[:], e_all[b][:], axis=X)
            nc.vector.reciprocal(rinv[:], s_sum[:])
            out_f = fin.tile([P, HID], f32, tag="outf", name="outf")
            if b < BT - 1:
                nc.scalar.activation(out_f[:], out_acc[b][:], Copy, scale=rinv[:])
                nc.sync.dma_start(out_d[b * P:(b + 1) * P, :], out_f[:])
            else:
                # last tile: DVE is idle by now and faster than the ScalarE
                # copy; chunked so the first DMA overlaps the second multiply
                for n in range(NC_H):
                    ncol = slice(n * NCHUNK, (n + 1) * NCHUNK)
                    nc.vector.tensor_scalar_mul(out_f[:, ncol],
                                                out_acc[b][:, ncol], rinv[:])
                    nc.sync.dma_start(out_d[b * P:(b + 1) * P, ncol],
                                      out_f[:, ncol])

    nc.compile()
    return nc


def _get_nc():
    if "nc" not in _cache:
        _cache["nc"] = _build()
    return _cache["nc"]


def build_in_maps(encoder_input, decoder_input, W_enc, b_enc, W_heads,
                  b_heads, W_dec, b_dec):
    import ml_dtypes
    bf = ml_dtypes.bfloat16
    f32c = lambda a: np.asarray(a, dtype=np.float32)
    cast = lambda a: np.ascontiguousarray(a, dtype=np.float32).astype(bf)

    KT_E, KT_D, KT_H = ENC_DIM // P, DEC_DIM // P, HID // P

    xeT = f32c(encoder_input).T                     # [1024, 4096]
    xdT = f32c(decoder_input).T                     # [512, 4096]
    W_enc = f32c(W_enc)                             # [1024, 1024]
    W_dec = f32c(W_dec)                             # [512, 1024]
    W_heads = f32c(W_heads)                         # [8, 1024, 1024]

    # w_enc_r[w, i][p, k4*512 + c] = W_enc[(i*4+k4)*128+p, w*512 + c]
    we4 = W_enc.reshape(KT_E, P, 2, HID // 2)       # [k, p, wave, c]
    we_r = np.zeros((2, 2, P, (KT_E // 2) * (HID // 2)), np.float32)
    for w in range(2):
        for i in range(2):
            blk = we4[i * 4:(i + 1) * 4, :, w, :]   # [4k, 128, 512]
            we_r[w, i] = blk.transpose(1, 0, 2).reshape(P, -1)
    xe4 = xeT.reshape(KT_E, P, BATCH)
    wd4 = W_dec.reshape(KT_D, P, HID)
    wd_r = np.ascontiguousarray(wd4.transpose(1, 0, 2).reshape(P, -1))
    wh4 = W_heads.reshape(HEADS, KT_H, P, HID)
    wh_r = np.ascontiguousarray(wh4.transpose(0, 2, 1, 3).reshape(HEADS, P, -1))

    bh_bc = np.broadcast_to(f32c(b_heads)[:, None, :], (HEADS, P, HID))
    bd_bc = np.broadcast_to(f32c(b_dec)[None, :], (P, HID))
    shared = {
        "w_enc_r": cast(we_r),
        "b_enc_pp": np.ascontiguousarray(f32c(b_enc).reshape(HID // P, P).T),
        "w_heads_r": cast(wh_r),
        "b_heads_bc": cast(bh_bc),
        "w_dec_r": cast(wd_r),
        "b_dec_bc": cast(bd_bc),
    }
    xd4 = xdT.reshape(KT_D, P, BATCH)
    in_maps = []
    for c in range(N_CORES):
        sl = slice(c * B_LOC, (c + 1) * B_LOC)
        m = dict(shared)
        xe_c = xe4[:, :, sl]                        # [8k, 128, 512]
        m["x_enc_r"] = cast(np.stack(
            [xe_c[i * 4:(i + 1) * 4].transpose(1, 0, 2).reshape(P, -1)
             for i in range(2)]))
        m["x_dec_r"] = cast(xd4[:, :, sl].transpose(1, 0, 2).reshape(P, -1))
        in_maps.append(m)
    return in_maps


def kernel(encoder_input, decoder_input, W_enc, b_enc, W_heads, b_heads,
           W_dec, b_dec):
    from concourse.bass_utils import run_bass_kernel_spmd

    nc = _get_nc()
    in_maps = build_in_maps(encoder_input, decoder_input, W_enc, b_enc,
                            W_heads, b_heads, W_dec, b_dec)
    res = run_bass_kernel_spmd(nc, in_maps, list(range(N_CORES)))
    out = np.concatenate([res.results[c]["out"] for c in range(N_CORES)], axis=0)
    return out.astype(np.float32)
